# revision 54
# baseline (speedup 1.0000x reference)
"""NetVLAD Trainium2 kernel — data-parallel over N across 8 cores.

Host side: x is uploaded as bf16 (halves DMA traffic); tiny consts are
packed per dtype (bf16 weights/identity, fp32 bias/centroids/ones).

Per core (4 images, 16 chunks of 1024 px, 8 tiles of 128 px each):

Phase 0 (norms): x DMA'd in half-image slabs; squares (DVE/ACT/gpsimd
split) feed 8 ssq matmuls per chunk into one PSUM bank; pixel norms for
images 0-1 come from one batched ACT Sqrt + DVE reciprocal (a single
sqrt-table round trip), images 2-3 are produced the same way mid-loop.

Phase 1 (softmax + vlad), dual-stream: chunks of two images interleave
(img0/img1, then img2/img3) so each stream's serial softmax chain fills
the other's bubbles.  Per chunk:
  PE:   logits psl[px,k] = xb_t.T @ wT, transpose pst[px,c] = xb_t.T @ I
        (bf16, per 128-px tile); ACT evicts pst -> xTs slabs [px,(8,129)]
        bf16 with n in column 128.
  DVE:  lu = raw*inv_n (bcast), negm = -max_k, dd = ll + negm (fp32),
        scol = sum_k exp, r = inv_n/sumexp.
  gpsimd: ll = lu + bias, aa = ee * r (bf16).
  ACT:  ee = exp(dd) -> bf16.
  PE:   psV[56,0:129] += aa_t[:, :56].T @ xTs_t  (bf16, accum per image).
Tails run in the psV bank (double-buffered so the next image overlaps):
vk = term1 - s*cen, bf16 transposes, Square-accum norms, rsqrt via
gpsimd pow / a Newton step around ||U||_F^2 ~= 128, final DMA out.
"""

import sys

for _p in ("/opt/trn_rl_repo",):
    if _p not in sys.path:
        sys.path.insert(0, _p)

import numpy as np

NIMG = 4      # images per core
C = 128
K = 64
KE = 56
P = 4096
TPC = 8       # 128-px tiles per chunk
CH = TPC * 128
NCH = P // CH           # 4 chunks per image
NT = NIMG * NCH         # 16 chunks per core

_cache = {}


def _build():
    import concourse.bass as bass
    import concourse.mybir as mybir
    from concourse import bacc, tile

    f32 = mybir.dt.float32
    f16 = mybir.dt.float16
    bf16 = mybir.dt.bfloat16
    Alu = mybir.AluOpType
    Act = mybir.ActivationFunctionType
    AxX = mybir.AxisListType.X

    nc = bacc.Bacc()
    x_in = nc.declare_dram_parameter("xb", [NIMG, C, P], bf16, isOutput=False)
    # cstb bf16 [C, 193]: 0:64 wT | 64:192 ident | 192 ones
    cb_in = nc.declare_dram_parameter("cstb", [C, 193], bf16, isOutput=False)
    # csth fp32 [C, 512]: conv_b tiled 8x
    ch_in = nc.declare_dram_parameter("csth", [C, 512], f32, isOutput=False)
    # cstf fp32 [C, 424]: 0:128 ident | 128:256 cen(rows 0:56) | 256 ones-col
    # | 258:386 ones-row (row 0) | 392:424 = -0.5 block
    cf_in = nc.declare_dram_parameter("cstf", [C, 424], f32, isOutput=False)
    out_ext = nc.declare_dram_parameter("out", [NIMG, KE, C], f32, isOutput=True)

    with tile.TileContext(nc, pool_alloc_mode="queue") as tc:
        with (
            tc.tile_pool(name="const", bufs=1) as cpool,
            tc.tile_pool(name="xin", bufs=9) as xpool,
            tc.tile_pool(name="work", bufs=2) as wpool,
            tc.tile_pool(name="stats", bufs=3) as spool,
            tc.tile_pool(name="fin", bufs=3) as fpool,
            tc.tile_pool(name="psL", bufs=2, space="PSUM") as pL,
            tc.tile_pool(name="psT", bufs=1, space="PSUM") as pT,
            tc.tile_pool(name="psS", bufs=1, space="PSUM") as pS,
            tc.tile_pool(name="psV", bufs=2, space="PSUM") as pV,
        ):
            cstb = cpool.tile([C, 193], bf16, tag="cstb")
            csth = cpool.tile([C, 512], f32, tag="csth")
            cstf = cpool.tile([C, 424], f32, tag="cstf")
            wTb = cstb[:, 0:K]
            identb = cstb[:, K:K + C]
            onesb = cstb[:, 192:193]
            b8h = csth[:]                 # fp16 bias, tiled 8x
            identf = cstf[:, 0:128]
            cen = cstf[0:KE, 128:256]
            onesf = cstf[:, 256:257]
            onesrow = cstf[0:1, 258:386]
            neghalf = cstf[:, 392:424]

            state = {"dma": {}, "xsq": {}}

            def emit_dma(half):
                # one DMA per half-image (2 chunks); alternate issue engine.
                # The very first half arrives as two quarter transfers so
                # chunk 0's squares/ssq matmuls start ~1 us earlier.
                if half >= 2 * NIMG or half in state["dma"]:
                    return
                img, hh = divmod(half, 2)
                xb = xpool.tile([C, P // 2], bf16, tag="x", name="xb")
                eng = nc.sync if half % 2 == 0 else nc.scalar
                if half == 0:
                    eng.dma_start(xb[:, 0:CH], x_in[img, :, 0:CH])
                    eng.dma_start(xb[:, CH:2 * CH], x_in[img, :, CH:2 * CH])
                else:
                    eng.dma_start(xb[:], x_in[img, :, hh * (P // 2):
                                              (hh + 1) * (P // 2)])
                state["dma"][half] = xb

            def xb_chunk(t):
                half, ch = divmod(t, NCH // 2)
                return state["dma"][half][:, ch * CH:(ch + 1) * CH]

            def emit_xsq(t):
                if t >= NT or t in state["xsq"]:
                    return
                emit_dma(t)
                xb = state["dma"][t]
                xsq = wpool.tile([C, CH], bf16, tag="xsq", name="xsq",
                                 bufs=4)
                nc.scalar.activation(xsq[:], xb[:], Act.Square)
                state["xsq"][t] = xsq

            def emit_ssqmm(t, engine):
                """Prologue: squares (split across engines) + ssq matmuls
                into one per-core PSUM bank."""
                xb = xb_chunk(t)
                xsq = wpool.tile([C, CH], bf16, tag="xsq", name="xsq",
                                 bufs=4)
                if engine == "dve":
                    nc.vector.tensor_mul(xsq[:], xb, xb)
                elif engine == "gp":
                    nc.gpsimd.tensor_mul(xsq[:], xb, xb)
                else:
                    nc.scalar.activation(xsq[:], xb, Act.Square)
                pss = state["pss"]
                for j in range(TPC):
                    nc.tensor.matmul(pss[:, t * 8 + j:t * 8 + j + 1],
                                     xsq[:, j * 128:(j + 1) * 128], onesb,
                                     start=True, stop=True)

            def norm_tiles():
                if "norms" not in state:
                    state["ssqs"] = spool.tile([C, 128], f32, tag="ssqs",
                                               name="ssqs")
                    state["nall"] = spool.tile([C, 128], f32, tag="nall",
                                               name="nall")
                    state["invc"] = spool.tile([C, 128], f32, tag="invc",
                                               name="invc")
                    state["norms"] = (state["invc"], state["nall"])
                return state["ssqs"], state["nall"], state["invc"]

            def emit_pow_chunk(t):
                """Early-chunk norms via gpsimd pow (no activation-table
                traffic; runs in the idle prologue gpsimd)."""
                ssqs, nall, invc = norm_tiles()
                lo, hi = t * 8, t * 8 + 8
                pss = state["pss"]
                nc.vector.tensor_copy(ssqs[:, lo:hi], pss[:, lo:hi])
                nc.gpsimd.tensor_tensor(invc[:, lo:hi], ssqs[:, lo:hi],
                                        neghalf[:, 0:8], Alu.pow)
                nc.vector.tensor_tensor(nall[:, lo:hi], ssqs[:, lo:hi],
                                        invc[:, lo:hi], Alu.mult)

            def emit_norms_rest(ranges):
                """Batched rsqrt for the remaining chunks: ACT Sqrt + DVE
                reciprocal (one sqrt-table round trip)."""
                ssqs, nall, invc = norm_tiles()
                pss = state["pss"]
                for lo, hi in ranges:
                    nc.vector.tensor_copy(ssqs[:, lo:hi], pss[:, lo:hi])
                    nc.scalar.activation(nall[:, lo:hi], ssqs[:, lo:hi],
                                         Act.Sqrt)
                    nc.vector.reciprocal(invc[:, lo:hi], nall[:, lo:hi])

            def emit_f1(t):
                """Logits + transpose matmuls and the xT eviction of chunk t
                (one chunk ahead)."""
                if t >= NT:
                    return
                xb = xb_chunk(t)
                psl = pL.tile([C, TPC * K], f32, tag="L", name="psl")
                for j in range(TPC):
                    nc.tensor.matmul(psl[:, j * K:(j + 1) * K],
                                     xb[:, j * 128:(j + 1) * 128], wTb,
                                     start=True, stop=True)
                # transposes into two half-tiles (1 bank each, ring 3) with
                # eviction right after each half -> [x | n] slabs (bf16)
                xTs = wpool.tile([C, TPC * 129], bf16, tag="xTs", name="xTs",
                                 bufs=6)
                xv = xTs[:].rearrange("p (t q) -> p t q", q=129)
                h = TPC // 2
                for half in range(2):
                    pst = pT.tile([C, h * 128], f32, tag="T", name="pst",
                                  bufs=3)
                    for jj in range(h):
                        j = half * h + jj
                        nc.tensor.matmul(pst[:, jj * 128:(jj + 1) * 128],
                                         xb[:, j * 128:(j + 1) * 128],
                                         identb, start=True, stop=True)
                    pv_ = pst[:].rearrange("p (t q) -> p t q", q=128)
                    nc.scalar.activation(
                        xv[:, half * h:(half + 1) * h, 0:128], pv_,
                        Act.Copy)
                state[("f1", t)] = (psl, xTs, xv)

            l3 = lambda tl, q: tl[:].rearrange("p (t k) -> p t k", k=q)

            def emit_lu(t):
                invc_i, _ = state["norms"]
                invc = invc_i[:, t * 8:t * 8 + 8]
                psl, xTs, xv = state.pop(("f1", t))
                lu = wpool.tile([C, TPC * K], f32, tag="lu", bufs=12)
                nc.vector.tensor_tensor(
                    l3(lu, K), psl[:].rearrange("p (t k) -> p t k", k=K),
                    invc.broadcast_to([C, TPC, K]), Alu.mult)
                state[("b1", t)] = (lu, xTs, xv, invc)

            def emit_softmax(t):
                """ll (gpsimd), negm, dd, exp of chunk t."""
                lu, xTs, xv, invc = state.pop(("b1", t))
                ll = wpool.tile([C, TPC * K], f32, tag="ll", bufs=12)
                nc.gpsimd.tensor_tensor(ll[:], lu[:], b8h, Alu.add)
                negm = spool.tile([C, 8], f32, tag="negm", bufs=12)
                nc.vector.tensor_reduce(negm[:], l3(ll, K), axis=AxX,
                                        op=Alu.max, negate=True)
                dd = wpool.tile([C, TPC * K], f32, tag="dd", bufs=12)
                nc.vector.tensor_tensor(
                    l3(dd, K), l3(ll, K), negm[:].broadcast_to([C, TPC, K]),
                    Alu.add)
                ee = wpool.tile([C, TPC * K], bf16, tag="ee", bufs=12)
                nc.scalar.activation(ee[:], dd[:], Act.Exp)
                state[("sm", t)] = (ee, xTs, xv, invc)

            def emit_b2a(t):
                """sumexp + weight scale of chunk t (all inputs ready)."""
                if t < 0:
                    return
                ee, xTs, xv, invc = state.pop(("sm", t))
                scol = spool.tile([C, 8], f32, tag="scol", bufs=12)
                nc.vector.tensor_reduce(scol[:], l3(ee, K), axis=AxX,
                                        op=Alu.add)
                gcol = spool.tile([C, 8], f32, tag="gcol", bufs=12)
                nc.vector.reciprocal(gcol[:], scol[:])
                rcol = spool.tile([C, 8], bf16, tag="rcol", bufs=12)
                nc.vector.tensor_tensor(rcol[:], invc, gcol[:], Alu.mult)
                state[("b2", t)] = (ee, xTs, xv, rcol)

            def emit_b2b(t):
                """aa, n column, vlad accumulation, tail of chunk t."""
                if t < 0:
                    return
                img, ch = divmod(t, NCH)
                ee, xTs, xv, rcol = state.pop(("b2", t))
                aa = wpool.tile([C, TPC * KE], bf16, tag="aa", bufs=12)
                ev = ee[:].rearrange("p (t k) -> p t k", k=K)
                nc.gpsimd.tensor_tensor(
                    l3(aa, KE), ev[:, :, 0:KE],
                    rcol[:].broadcast_to([C, TPC, KE]), Alu.mult)
                _, nall_i = state["norms"]
                sv3 = nall_i[:, t * 8:t * 8 + 8].rearrange(
                    "p (t o) -> p t o", o=1)
                nc.scalar.activation(xv[:, :, 128:129], sv3, Act.Copy)

                if ch == 0:
                    state[("psV", img)] = pV.tile([C, 512], f32, tag="psV",
                                                  name="psv")
                psv = state[("psV", img)]
                for j in range(TPC):
                    nc.tensor.matmul(psv[0:KE, 0:129],
                                     aa[:, j * KE:(j + 1) * KE],
                                     xTs[:, j * 129:(j + 1) * 129],
                                     start=(ch == 0 and j == 0),
                                     stop=(ch == NCH - 1 and j == TPC - 1))
                if ch == NCH - 1:
                    emit_tail(img, state.pop(("psV", img)))

            def emit_tail(img, psv):
                # vk = term1 - s*cen  [56, 128] bf16
                negs = spool.tile([KE, 1], f32, tag="negs")
                nc.vector.tensor_scalar_mul(negs[:], psv[0:KE, 128:129], -1.0)
                vk = fpool.tile([KE, C], bf16, tag="vk")
                nc.vector.scalar_tensor_tensor(vk[:], cen, negs[:],
                                               psv[0:KE, 0:C],
                                               Alu.mult, Alu.add)
                # transpose -> [c, k] (bf16) into the same psV bank
                vkT = psv[:, 160:160 + KE].bitcast(bf16)[:, 0:KE]
                nc.tensor.matmul(vkT, vk[:], identb[0:KE, 0:KE],
                                 is_transpose=True, start=True, stop=True)
                trash = fpool.tile([C, KE], bf16, tag="trash")
                ssqk = spool.tile([C, 1], f32, tag="ssqk")
                nc.scalar.activation(trash[:], vkT, Act.Square,
                                     accum_out=ssqk[:])
                ssqc = spool.tile([C, 1], f32, tag="ssqc")
                nc.vector.tensor_scalar_max(ssqc[:], ssqk[:], 1e-24)
                invk = spool.tile([C, 1], f32, tag="invk")
                nc.gpsimd.tensor_tensor(invk[:], ssqc[:], neghalf[:, 0:1],
                                        Alu.pow)
                # q = ssqk * invk^2 -> 1 per live column; tot = sum_c q ~ 128
                iv2 = spool.tile([C, 1], f32, tag="iv2")
                nc.vector.tensor_tensor(iv2[:], invk[:], invk[:], Alu.mult)
                qv = spool.tile([C, 1], f32, tag="qv")
                nc.vector.tensor_tensor(qv[:], ssqc[:], iv2[:], Alu.mult)
                nc.tensor.matmul(psv[0:1, 288:289], qv[:], onesf,
                                 start=True, stop=True)
                # fv = rsqrt(tot) via one Newton step around tot ~= 128:
                # fv = (1.5 - tot/256) / sqrt(128)
                fv = spool.tile([1, 1], f32, tag="fv")
                nc.vector.tensor_scalar(fv[:], psv[0:1, 288:289],
                                        -1.0 / 256.0 / (128.0 ** 0.5),
                                        1.5 / (128.0 ** 0.5),
                                        Alu.mult, Alu.add)
                # broadcast fv to all partitions, comb = invk * fv
                nc.tensor.matmul(psv[:, 290:291], onesrow, fv[:],
                                 start=True, stop=True)
                comb = spool.tile([C, 1], f32, tag="comb")
                nc.vector.tensor_tensor(comb[:], invk[:], psv[:, 290:291],
                                        Alu.mult)
                obT = fpool.tile([C, KE], bf16, tag="obT")
                nc.vector.tensor_scalar(obT[:], vkT, comb[:], None, Alu.mult)
                # transpose back -> [k, c] (bf16), evict fp32, DMA out
                obb = psv[:, 320:384].bitcast(bf16)[0:KE, 0:C]
                nc.tensor.matmul(obb, obT[:], identb,
                                 is_transpose=True, start=True, stop=True)
                ob = fpool.tile([KE, C], f32, tag="ob")
                nc.scalar.activation(ob[:], obb, Act.Copy)
                nc.sync.dma_start(out_ext[img], ob[:])

            # Phase 0: x DMAs first, then squares + ssq matmuls for
            # images 0-1 and their batched norms; images 2-3 norms are
            # produced during the first half of the main loop.
            state["pss"] = pS.tile([C, 128], f32, tag="S", name="pss")
            emit_dma(0)
            emit_dma(1)
            nc.sync.dma_start(cstb[:], cb_in[:])
            emit_dma(2)
            nc.sync.dma_start(csth[:], ch_in[:])
            emit_dma(3)
            nc.sync.dma_start(cstf[:], cf_in[:])
            xsq_eng = ["dve", "act", "dve", "gp", "dve", "act", "dve", "act",
                       "act", "gp", "act", "gp", "act", "gp", "act", "act"]
            for c in (0, 4, 1, 5, 2, 6, 3, 7):
                emit_ssqmm(c, xsq_eng[c])
            emit_norms_rest([(0, 64)])
            for half in range(4, 8):
                emit_dma(half)
            # Phase 1: dual-stream softmax+vlad (img0/img1 then img2/img3)
            seq = [0, 4, 1, 5, 2, 6, 3, 7, 8, 12, 9, 13, 10, 14, 11, 15]
            emit_f1(seq[0])
            for i in range(NT):
                t = seq[i]
                if i < 8:
                    emit_ssqmm(8 + i, xsq_eng[8 + i])
                    if i == 7:
                        emit_norms_rest([(64, 128)])
                emit_b2a(seq[i - 1] if i > 0 else -1)
                if i + 1 < NT:
                    emit_f1(seq[i + 1])
                emit_lu(t)
                emit_b2b(seq[i - 1] if i > 0 else -1)
                emit_softmax(t)
            emit_b2a(seq[NT - 1])
            emit_b2b(seq[NT - 1])

    nc.compile()
    return nc


def _get_nc():
    if "nc" not in _cache:
        _cache["nc"] = _build()
    return _cache["nc"]


def _make_in_maps(inputs):
    import ml_dtypes

    x = np.asarray(inputs["x"], dtype=np.float32)
    conv_w = np.asarray(inputs["conv_w"], dtype=np.float32)
    conv_b = np.asarray(inputs["conv_b"], dtype=np.float32)
    centroids = np.asarray(inputs["centroids"], dtype=np.float32)

    N = x.shape[0]
    n_cores = 8
    per = N // n_cores
    assert per == NIMG

    xb = x.reshape(N, C, P).astype(ml_dtypes.bfloat16)

    cstb = np.zeros((C, 193), dtype=ml_dtypes.bfloat16)
    cstb[:, 0:K] = conv_w.T.astype(ml_dtypes.bfloat16)
    cstb[:, K:K + C] = np.eye(C, dtype=np.float32)
    cstb[:, 192] = 1.0

    csth = np.tile(conv_b.astype(np.float32), TPC)[None, :].repeat(C, axis=0)
    csth = np.ascontiguousarray(csth)

    cstf = np.zeros((C, 424), dtype=np.float32)
    cstf[:, 0:C] = np.eye(C, dtype=np.float32)
    cstf[0:KE, C:C + C] = centroids[:KE]
    cstf[:, 256] = 1.0
    cstf[0, 258:386] = 1.0
    cstf[:, 392:424] = -0.5

    in_maps = []
    for i in range(n_cores):
        in_maps.append({
            "xb": np.ascontiguousarray(xb[i * per:(i + 1) * per]),
            "cstb": cstb,
            "csth": csth,
            "cstf": cstf,
        })
    return in_maps


def kernel(x, conv_w, conv_b, centroids):
    from concourse.bass_utils import run_bass_kernel_spmd

    in_maps = _make_in_maps(
        {"x": x, "conv_w": conv_w, "conv_b": conv_b, "centroids": centroids}
    )
    nc = _get_nc()
    res = run_bass_kernel_spmd(nc, in_maps, list(range(8)))
    outs = [np.asarray(r["out"]).reshape(NIMG, KE * C) for r in res.results]
    return np.concatenate(outs, axis=0)


if __name__ == "__main__":
    rng = np.random.default_rng(0)
    x = rng.standard_normal((32, C, 64, 64), dtype=np.float32)
    w = rng.standard_normal((K, C), dtype=np.float32)
    b = rng.standard_normal((K,), dtype=np.float32)
    c = rng.random((K, C), dtype=np.float32)
    out = kernel(x=x, conv_w=w, conv_b=b, centroids=c)
    print(out.shape, out.dtype)


# revision 55
# speedup vs baseline: 1.0191x; 1.0191x over previous
"""NetVLAD Trainium2 kernel — data-parallel over N across 8 cores.

Host side: x is uploaded as bf16 (halves DMA traffic); tiny consts are
packed per dtype (bf16 weights/identity, fp32 bias/centroids/ones).

Per core (4 images, 16 chunks of 1024 px, 8 tiles of 128 px each):

Phase 0 (norms): x DMA'd in half-image slabs; squares (DVE/ACT/gpsimd
split) feed 8 ssq matmuls per chunk into one PSUM bank; pixel norms for
images 0-1 come from one batched ACT Sqrt + DVE reciprocal (a single
sqrt-table round trip), images 2-3 are produced the same way mid-loop.

Phase 1 (softmax + vlad), dual-stream: chunks of two images interleave
(img0/img1, then img2/img3) so each stream's serial softmax chain fills
the other's bubbles.  Per chunk:
  PE:   logits psl[px,k] = xb_t.T @ wT, transpose pst[px,c] = xb_t.T @ I
        (bf16, per 128-px tile); ACT evicts pst -> xTs slabs [px,(8,129)]
        bf16 with n in column 128.
  DVE:  lu = raw*inv_n (bcast), negm = -max_k, dd = ll + negm (fp32),
        scol = sum_k exp, r = inv_n/sumexp.
  gpsimd: ll = lu + bias, aa = ee * r (bf16).
  ACT:  ee = exp(dd) -> bf16.
  PE:   psV[56,0:129] += aa_t[:, :56].T @ xTs_t  (bf16, accum per image).
Tails run in the psV bank (double-buffered so the next image overlaps):
vk = term1 - s*cen, bf16 transposes, Square-accum norms, rsqrt via
gpsimd pow / a Newton step around ||U||_F^2 ~= 128, final DMA out.
"""

import sys

for _p in ("/opt/trn_rl_repo",):
    if _p not in sys.path:
        sys.path.insert(0, _p)

import numpy as np

NIMG = 4      # images per core
C = 128
K = 64
KE = 56
P = 4096
TPC = 8       # 128-px tiles per chunk
CH = TPC * 128
NCH = P // CH           # 4 chunks per image
NT = NIMG * NCH         # 16 chunks per core

_cache = {}


def _build():
    import concourse.bass as bass
    import concourse.mybir as mybir
    from concourse import bacc, tile

    f32 = mybir.dt.float32
    f16 = mybir.dt.float16
    bf16 = mybir.dt.bfloat16
    Alu = mybir.AluOpType
    Act = mybir.ActivationFunctionType
    AxX = mybir.AxisListType.X

    nc = bacc.Bacc()
    x_in = nc.declare_dram_parameter("xb", [NIMG, C, P], bf16, isOutput=False)
    # cstb bf16 [C, 193]: 0:64 wT | 64:192 ident | 192 ones
    cb_in = nc.declare_dram_parameter("cstb", [C, 193], bf16, isOutput=False)
    # csth fp32 [C, 512]: conv_b tiled 8x
    ch_in = nc.declare_dram_parameter("csth", [C, 512], f32, isOutput=False)
    # cstf fp32 [C, 424]: 0:128 ident | 128:256 cen(rows 0:56) | 256 ones-col
    # | 258:386 ones-row (row 0) | 392:424 = -0.5 block
    cf_in = nc.declare_dram_parameter("cstf", [C, 424], f32, isOutput=False)
    out_ext = nc.declare_dram_parameter("out", [NIMG, KE, C], f32, isOutput=True)

    with tile.TileContext(nc, pool_alloc_mode="queue") as tc:
        with (
            tc.tile_pool(name="const", bufs=1) as cpool,
            tc.tile_pool(name="xin", bufs=9) as xpool,
            tc.tile_pool(name="work", bufs=2) as wpool,
            tc.tile_pool(name="stats", bufs=2) as spool,
            tc.tile_pool(name="fin", bufs=2) as fpool,
            tc.tile_pool(name="psL", bufs=2, space="PSUM") as pL,
            tc.tile_pool(name="psT", bufs=1, space="PSUM") as pT,
            tc.tile_pool(name="psS", bufs=1, space="PSUM") as pS,
            tc.tile_pool(name="psV", bufs=2, space="PSUM") as pV,
        ):
            cstb = cpool.tile([C, 193], bf16, tag="cstb")
            csth = cpool.tile([C, 512], f32, tag="csth")
            cstf = cpool.tile([C, 424], f32, tag="cstf")
            wTb = cstb[:, 0:K]
            identb = cstb[:, K:K + C]
            onesb = cstb[:, 192:193]
            b8h = csth[:]                 # fp16 bias, tiled 8x
            identf = cstf[:, 0:128]
            cen = cstf[0:KE, 128:256]
            onesf = cstf[:, 256:257]
            onesrow = cstf[0:1, 258:386]
            neghalf = cstf[:, 392:424]

            state = {"dma": {}, "xsq": {}}

            def emit_dma(half):
                # one DMA per half-image (2 chunks); alternate issue engine
                if half >= 2 * NIMG or half in state["dma"]:
                    return
                img, hh = divmod(half, 2)
                xb = xpool.tile([C, P // 2], bf16, tag="x", name="xb")
                eng = nc.sync if half % 2 == 0 else nc.scalar
                eng.dma_start(xb[:], x_in[img, :, hh * (P // 2):
                                          (hh + 1) * (P // 2)])
                state["dma"][half] = xb

            def xb_chunk(t):
                half, ch = divmod(t, NCH // 2)
                return state["dma"][half][:, ch * CH:(ch + 1) * CH]

            def emit_xsq(t):
                if t >= NT or t in state["xsq"]:
                    return
                emit_dma(t)
                xb = state["dma"][t]
                xsq = wpool.tile([C, CH], bf16, tag="xsq", name="xsq",
                                 bufs=4)
                nc.scalar.activation(xsq[:], xb[:], Act.Square)
                state["xsq"][t] = xsq

            def emit_ssqmm(t, engine):
                """Prologue: squares (split across engines) + ssq matmuls
                into one per-core PSUM bank."""
                xb = xb_chunk(t)
                xsq = wpool.tile([C, CH], bf16, tag="xsq", name="xsq",
                                 bufs=4)
                if engine == "dve":
                    nc.vector.tensor_mul(xsq[:], xb, xb)
                elif engine == "gp":
                    nc.gpsimd.tensor_mul(xsq[:], xb, xb)
                else:
                    nc.scalar.activation(xsq[:], xb, Act.Square)
                pss = state["pss"]
                for j in range(TPC):
                    nc.tensor.matmul(pss[:, t * 8 + j:t * 8 + j + 1],
                                     xsq[:, j * 128:(j + 1) * 128], onesb,
                                     start=True, stop=True)

            def norm_tiles():
                if "norms" not in state:
                    state["ssqs"] = spool.tile([C, 128], f32, tag="ssqs",
                                               name="ssqs")
                    state["nall"] = spool.tile([C, 128], f32, tag="nall",
                                               name="nall")
                    state["invc"] = spool.tile([C, 128], f32, tag="invc",
                                               name="invc")
                    state["norms"] = (state["invc"], state["nall"])
                return state["ssqs"], state["nall"], state["invc"]

            def emit_pow_chunk(t):
                """Early-chunk norms via gpsimd pow (no activation-table
                traffic; runs in the idle prologue gpsimd)."""
                ssqs, nall, invc = norm_tiles()
                lo, hi = t * 8, t * 8 + 8
                pss = state["pss"]
                nc.vector.tensor_copy(ssqs[:, lo:hi], pss[:, lo:hi])
                nc.gpsimd.tensor_tensor(invc[:, lo:hi], ssqs[:, lo:hi],
                                        neghalf[:, 0:8], Alu.pow)
                nc.vector.tensor_tensor(nall[:, lo:hi], ssqs[:, lo:hi],
                                        invc[:, lo:hi], Alu.mult)

            def emit_norms_rest(ranges):
                """Batched rsqrt for the remaining chunks: ACT Sqrt + DVE
                reciprocal (one sqrt-table round trip)."""
                ssqs, nall, invc = norm_tiles()
                pss = state["pss"]
                for lo, hi in ranges:
                    nc.vector.tensor_copy(ssqs[:, lo:hi], pss[:, lo:hi])
                    nc.scalar.activation(nall[:, lo:hi], ssqs[:, lo:hi],
                                         Act.Sqrt)
                    nc.vector.reciprocal(invc[:, lo:hi], nall[:, lo:hi])

            def emit_f1(t):
                """Logits + transpose matmuls and the xT eviction of chunk t
                (one chunk ahead)."""
                if t >= NT:
                    return
                xb = xb_chunk(t)
                psl = pL.tile([C, TPC * K], f32, tag="L", name="psl")
                for j in range(TPC):
                    nc.tensor.matmul(psl[:, j * K:(j + 1) * K],
                                     xb[:, j * 128:(j + 1) * 128], wTb,
                                     start=True, stop=True)
                # transposes into two half-tiles (1 bank each, ring 3) with
                # eviction right after each half -> [x | n] slabs (bf16)
                xTs = wpool.tile([C, TPC * 129], bf16, tag="xTs", name="xTs",
                                 bufs=6)
                xv = xTs[:].rearrange("p (t q) -> p t q", q=129)
                h = TPC // 2
                for half in range(2):
                    pst = pT.tile([C, h * 128], f32, tag="T", name="pst",
                                  bufs=3)
                    for jj in range(h):
                        j = half * h + jj
                        nc.tensor.matmul(pst[:, jj * 128:(jj + 1) * 128],
                                         xb[:, j * 128:(j + 1) * 128],
                                         identb, start=True, stop=True)
                    pv_ = pst[:].rearrange("p (t q) -> p t q", q=128)
                    nc.scalar.activation(
                        xv[:, half * h:(half + 1) * h, 0:128], pv_,
                        Act.Copy)
                state[("f1", t)] = (psl, xTs, xv)

            l3 = lambda tl, q: tl[:].rearrange("p (t k) -> p t k", k=q)

            def emit_lu(t):
                invc_i, _ = state["norms"]
                invc = invc_i[:, t * 8:t * 8 + 8]
                psl, xTs, xv = state.pop(("f1", t))
                lu = wpool.tile([C, TPC * K], f32, tag="lu", bufs=12)
                nc.vector.tensor_tensor(
                    l3(lu, K), psl[:].rearrange("p (t k) -> p t k", k=K),
                    invc.broadcast_to([C, TPC, K]), Alu.mult)
                state[("b1", t)] = (lu, xTs, xv, invc)

            def emit_softmax(t):
                """ll (gpsimd), negm, dd, exp of chunk t."""
                lu, xTs, xv, invc = state.pop(("b1", t))
                ll = wpool.tile([C, TPC * K], f32, tag="ll", bufs=12)
                nc.gpsimd.tensor_tensor(ll[:], lu[:], b8h, Alu.add)
                negm = spool.tile([C, 8], f32, tag="negm", bufs=12)
                nc.vector.tensor_reduce(negm[:], l3(ll, K), axis=AxX,
                                        op=Alu.max, negate=True)
                dd = wpool.tile([C, TPC * K], f32, tag="dd", bufs=12)
                nc.vector.tensor_tensor(
                    l3(dd, K), l3(ll, K), negm[:].broadcast_to([C, TPC, K]),
                    Alu.add)
                ee = wpool.tile([C, TPC * K], bf16, tag="ee", bufs=12)
                nc.scalar.activation(ee[:], dd[:], Act.Exp)
                state[("sm", t)] = (ee, xTs, xv, invc)

            def emit_b2a(t):
                """sumexp + weight scale of chunk t (all inputs ready)."""
                if t < 0:
                    return
                ee, xTs, xv, invc = state.pop(("sm", t))
                scol = spool.tile([C, 8], f32, tag="scol", bufs=12)
                nc.vector.tensor_reduce(scol[:], l3(ee, K), axis=AxX,
                                        op=Alu.add)
                gcol = spool.tile([C, 8], f32, tag="gcol", bufs=12)
                nc.vector.reciprocal(gcol[:], scol[:])
                rcol = spool.tile([C, 8], bf16, tag="rcol", bufs=12)
                nc.vector.tensor_tensor(rcol[:], invc, gcol[:], Alu.mult)
                state[("b2", t)] = (ee, xTs, xv, rcol)

            def emit_b2b(t):
                """aa, n column, vlad accumulation, tail of chunk t."""
                if t < 0:
                    return
                img, ch = divmod(t, NCH)
                ee, xTs, xv, rcol = state.pop(("b2", t))
                aa = wpool.tile([C, TPC * KE], bf16, tag="aa", bufs=12)
                ev = ee[:].rearrange("p (t k) -> p t k", k=K)
                nc.gpsimd.tensor_tensor(
                    l3(aa, KE), ev[:, :, 0:KE],
                    rcol[:].broadcast_to([C, TPC, KE]), Alu.mult)
                _, nall_i = state["norms"]
                sv3 = nall_i[:, t * 8:t * 8 + 8].rearrange(
                    "p (t o) -> p t o", o=1)
                nc.scalar.activation(xv[:, :, 128:129], sv3, Act.Copy)

                if ch == 0:
                    state[("psV", img)] = pV.tile([C, 512], f32, tag="psV",
                                                  name="psv")
                psv = state[("psV", img)]
                for j in range(TPC):
                    nc.tensor.matmul(psv[0:KE, 0:129],
                                     aa[:, j * KE:(j + 1) * KE],
                                     xTs[:, j * 129:(j + 1) * 129],
                                     start=(ch == 0 and j == 0),
                                     stop=(ch == NCH - 1 and j == TPC - 1))
                if ch == NCH - 1:
                    emit_tail(img, state.pop(("psV", img)))

            def emit_tail(img, psv):
                # vk = term1 - s*cen  [56, 128] bf16
                negs = spool.tile([KE, 1], f32, tag="negs")
                nc.vector.tensor_scalar_mul(negs[:], psv[0:KE, 128:129], -1.0)
                vk = fpool.tile([KE, C], bf16, tag="vk")
                nc.vector.scalar_tensor_tensor(vk[:], cen, negs[:],
                                               psv[0:KE, 0:C],
                                               Alu.mult, Alu.add)
                # transpose -> [c, k] (bf16) into the same psV bank
                vkT = psv[:, 160:160 + KE].bitcast(bf16)[:, 0:KE]
                nc.tensor.matmul(vkT, vk[:], identb[0:KE, 0:KE],
                                 is_transpose=True, start=True, stop=True)
                trash = fpool.tile([C, KE], bf16, tag="trash")
                ssqk = spool.tile([C, 1], f32, tag="ssqk")
                nc.scalar.activation(trash[:], vkT, Act.Square,
                                     accum_out=ssqk[:])
                ssqc = spool.tile([C, 1], f32, tag="ssqc")
                nc.vector.tensor_scalar_max(ssqc[:], ssqk[:], 1e-24)
                invk = spool.tile([C, 1], f32, tag="invk")
                nc.gpsimd.tensor_tensor(invk[:], ssqc[:], neghalf[:, 0:1],
                                        Alu.pow)
                # q = ssqk * invk^2 -> 1 per live column; tot = sum_c q ~ 128
                iv2 = spool.tile([C, 1], f32, tag="iv2")
                nc.vector.tensor_tensor(iv2[:], invk[:], invk[:], Alu.mult)
                qv = spool.tile([C, 1], f32, tag="qv")
                nc.vector.tensor_tensor(qv[:], ssqc[:], iv2[:], Alu.mult)
                nc.tensor.matmul(psv[0:1, 288:289], qv[:], onesf,
                                 start=True, stop=True)
                # fv = rsqrt(tot) via one Newton step around tot ~= 128:
                # fv = (1.5 - tot/256) / sqrt(128)
                fv = spool.tile([1, 1], f32, tag="fv")
                nc.vector.tensor_scalar(fv[:], psv[0:1, 288:289],
                                        -1.0 / 256.0 / (128.0 ** 0.5),
                                        1.5 / (128.0 ** 0.5),
                                        Alu.mult, Alu.add)
                # broadcast fv to all partitions, comb = invk * fv
                nc.tensor.matmul(psv[:, 290:291], onesrow, fv[:],
                                 start=True, stop=True)
                comb = spool.tile([C, 1], f32, tag="comb")
                nc.vector.tensor_tensor(comb[:], invk[:], psv[:, 290:291],
                                        Alu.mult)
                obT = fpool.tile([C, KE], bf16, tag="obT")
                nc.vector.tensor_scalar(obT[:], vkT, comb[:], None, Alu.mult)
                # transpose back -> [k, c] (bf16), evict fp32, DMA out
                obb = psv[:, 320:384].bitcast(bf16)[0:KE, 0:C]
                nc.tensor.matmul(obb, obT[:], identb,
                                 is_transpose=True, start=True, stop=True)
                ob = fpool.tile([KE, C], f32, tag="ob")
                nc.scalar.activation(ob[:], obb, Act.Copy)
                nc.sync.dma_start(out_ext[img], ob[:])

            # Phase 0: x DMAs first, then squares + ssq matmuls for
            # images 0-1 and their batched norms; images 2-3 norms are
            # produced during the first half of the main loop.
            state["pss"] = pS.tile([C, 128], f32, tag="S", name="pss")
            emit_dma(0)
            emit_dma(1)
            nc.sync.dma_start(cstb[:], cb_in[:])
            emit_dma(2)
            nc.sync.dma_start(csth[:], ch_in[:])
            emit_dma(3)
            nc.sync.dma_start(cstf[:], cf_in[:])
            xsq_eng = ["dve", "act", "dve", "gp", "dve", "act", "dve", "act",
                       "act", "gp", "act", "gp", "act", "gp", "act", "act"]
            for c in (0, 4, 1, 5, 2, 6, 3, 7):
                emit_ssqmm(c, xsq_eng[c])
            emit_norms_rest([(0, 64)])
            for half in range(4, 8):
                emit_dma(half)
            # Phase 1: dual-stream softmax+vlad (img0/img1 then img2/img3)
            seq = [0, 4, 1, 5, 2, 6, 3, 7, 8, 12, 9, 13, 10, 14, 11, 15]
            emit_f1(seq[0])
            for i in range(NT):
                t = seq[i]
                if i < 8:
                    emit_ssqmm(8 + i, xsq_eng[8 + i])
                    if i == 7:
                        emit_norms_rest([(64, 128)])
                emit_b2a(seq[i - 1] if i > 0 else -1)
                if i + 1 < NT:
                    emit_f1(seq[i + 1])
                emit_lu(t)
                emit_b2b(seq[i - 1] if i > 0 else -1)
                emit_softmax(t)
            emit_b2a(seq[NT - 1])
            emit_b2b(seq[NT - 1])

    nc.compile()
    return nc


def _get_nc():
    if "nc" not in _cache:
        _cache["nc"] = _build()
    return _cache["nc"]


def _make_in_maps(inputs):
    import ml_dtypes

    x = np.asarray(inputs["x"], dtype=np.float32)
    conv_w = np.asarray(inputs["conv_w"], dtype=np.float32)
    conv_b = np.asarray(inputs["conv_b"], dtype=np.float32)
    centroids = np.asarray(inputs["centroids"], dtype=np.float32)

    N = x.shape[0]
    n_cores = 8
    per = N // n_cores
    assert per == NIMG

    xb = x.reshape(N, C, P).astype(ml_dtypes.bfloat16)

    cstb = np.zeros((C, 193), dtype=ml_dtypes.bfloat16)
    cstb[:, 0:K] = conv_w.T.astype(ml_dtypes.bfloat16)
    cstb[:, K:K + C] = np.eye(C, dtype=np.float32)
    cstb[:, 192] = 1.0

    csth = np.tile(conv_b.astype(np.float32), TPC)[None, :].repeat(C, axis=0)
    csth = np.ascontiguousarray(csth)

    cstf = np.zeros((C, 424), dtype=np.float32)
    cstf[:, 0:C] = np.eye(C, dtype=np.float32)
    cstf[0:KE, C:C + C] = centroids[:KE]
    cstf[:, 256] = 1.0
    cstf[0, 258:386] = 1.0
    cstf[:, 392:424] = -0.5

    in_maps = []
    for i in range(n_cores):
        in_maps.append({
            "xb": np.ascontiguousarray(xb[i * per:(i + 1) * per]),
            "cstb": cstb,
            "csth": csth,
            "cstf": cstf,
        })
    return in_maps


def kernel(x, conv_w, conv_b, centroids):
    from concourse.bass_utils import run_bass_kernel_spmd

    in_maps = _make_in_maps(
        {"x": x, "conv_w": conv_w, "conv_b": conv_b, "centroids": centroids}
    )
    nc = _get_nc()
    res = run_bass_kernel_spmd(nc, in_maps, list(range(8)))
    outs = [np.asarray(r["out"]).reshape(NIMG, KE * C) for r in res.results]
    return np.concatenate(outs, axis=0)


if __name__ == "__main__":
    rng = np.random.default_rng(0)
    x = rng.standard_normal((32, C, 64, 64), dtype=np.float32)
    w = rng.standard_normal((K, C), dtype=np.float32)
    b = rng.standard_normal((K,), dtype=np.float32)
    c = rng.random((K, C), dtype=np.float32)
    out = kernel(x=x, conv_w=w, conv_b=b, centroids=c)
    print(out.shape, out.dtype)
